# revision 107
# baseline (speedup 1.0000x reference)
"""Trainium2 Bass kernel for nn_MultiHeadAttention_7834020348049.

Reference computation (per token, no cross-token interaction):
    qn  = LayerNorm(q) * gamma_m + beta_m
    kvn = LayerNorm(kv) * gamma_l + beta_l
    Q = qn @ Wq.T ; K,V = split(kvn @ Wkv.T)
    per token: scores[h,g] = Q[h,:] . K[g,:] / sqrt(128)  (8x8 over heads)
    ctx[h,:] = softmax_g(scores) @ V
    out = ctx @ Wo.T

Sharding: pure data-parallel over the 16*2048 = 32768 tokens -> 4096/core.

v2 pipeline (all-fp16 matmuls at 1 cycle/row, bf16 softmax for range):
  token-major LN (bn_stats/bn_aggr fp32 stats, fp16 data)
  -> PE transpose (fp16) to feature-major qn^T / kvn^T
  -> projections with weights stationary, 512-token chunks, per-head
     contiguous Q^T/K^T/V^T [d][h][t] (strided matmul-operand APs instead
     of strided writes)
  -> per 128-token tile, two 4-subtile batches:
     scores S[(t,h),(t,g)] via strided APs, batched exp (ACT, bf16),
     tensor_tensor_reduce mask+rowsum, reciprocal, tensor_scalar P (fp16),
     PE transposes of P and V (fp16 PSUM), ctx matmul, batched copies
  -> token-major O-projection (strided ctx^T head reads), fp16 output.
"""
import sys, os
sys.path.insert(0, "/opt/trn_rl_repo")
os.environ.setdefault("JAX_PLATFORMS", "cpu")

from contextlib import ExitStack
import numpy as np
import ml_dtypes

import concourse.bass as bass
import concourse.bacc as bacc
import concourse.tile as tile
from concourse import mybir
from concourse.masks import make_identity
from concourse.bass_utils import run_bass_kernel_spmd

F32 = mybir.dt.float32
F16 = mybir.dt.float16
BF16 = mybir.dt.bfloat16

DIM = 1024
HEADS = 8
DHEAD = 128
NCORES = 8

TC = 512   # tokens per chunk (projection moving-dim)
TT = 128   # tokens per tile (partition dim)
TS = 16    # tokens per attention sub-tile
KT_F = DIM // 128  # 8 k-tiles for the 1024-feature contraction


def head_windows(t, h, nwin):
    """head-h columns across nwin h-major sub-tile windows of an
    interleaved [128, nwin*128] tensor: window w holds cols
    w*128 + h*16 + t_local. Two free dims, 16-elem packed runs."""
    return bass.AP(tensor=t.tensor, offset=t.offset + h * TS,
                   ap=[t.ap[0], [128, nwin], [1, TS]])


def build_nc(T, with_bias_q=False, with_bias_kv=False):
    nc = bacc.Bacc(trn_type="TRN2", target_bir_lowering=False)

    q_d = nc.dram_tensor("q", [T, DIM], F16, kind="ExternalInput").ap()
    kv_d = nc.dram_tensor("kv", [T, DIM], F16, kind="ExternalInput").ap()
    wq_d = nc.dram_tensor("wq", [DIM, DIM], F16, kind="ExternalInput").ap()
    wkv_d = nc.dram_tensor("wkv", [DIM, 2 * DIM], F16, kind="ExternalInput").ap()
    wo_d = nc.dram_tensor("wo", [DIM, DIM], F16, kind="ExternalInput").ap()
    mask_d = nc.dram_tensor("mask", [TT, 4 * TT], BF16, kind="ExternalInput").ap()
    ident_d = nc.dram_tensor("ident", [128, 128], F16, kind="ExternalInput").ap()
    bq_d = bkv_d = None
    if with_bias_q:
        bq_d = nc.dram_tensor("bq", [1, DIM], F16, kind="ExternalInput").ap()
    if with_bias_kv:
        bkv_d = nc.dram_tensor("bkv", [1, 2 * DIM], F16, kind="ExternalInput").ap()
    out_d = nc.dram_tensor("out", [T, DIM], F16, kind="ExternalOutput").ap()

    NCH = T // TC        # chunks
    TPC = TC // TT       # tiles per chunk (4)
    SPT = TT // TS       # sub-tiles per tile (8)

    with tile.TileContext(nc) as tc, ExitStack() as ctx:
        # ---------------- static SBUF ----------------
        singles = ctx.enter_context(tc.tile_pool(name="singles", bufs=1))
        ident = singles.tile([128, 128], F16)
        nc.sync.dma_start(ident[:], ident_d)
        mask = singles.tile([TT, 4 * TT], BF16)
        nc.sync.dma_start(mask[:], mask_d)
        eps = singles.tile([128, 1], F32)
        nc.vector.memset(eps[:], 1e-5)

        wq_sb = singles.tile([128, KT_F, DIM], F16)
        wkv_sb = singles.tile([128, KT_F, 2 * DIM], F16)
        wo_sb = singles.tile([128, KT_F, DIM], F16)

        def emit_weight_loads():
            for k in range(KT_F):
                nc.sync.dma_start(wq_sb[:, k, :], wq_d[k * 128:(k + 1) * 128, :])
                nc.sync.dma_start(wkv_sb[:, k, :],
                                  wkv_d[k * 128:(k + 1) * 128, :])
                nc.sync.dma_start(wo_sb[:, k, :], wo_d[k * 128:(k + 1) * 128, :])
        if with_bias_q or with_bias_kv:
            ones_row = singles.tile([1, TC], F16)
            nc.vector.memset(ones_row[:], 1.0)
        if with_bias_q:
            bq_sb = singles.tile([1, DIM], F16)
            nc.sync.dma_start(bq_sb[:], bq_d)
        if with_bias_kv:
            bkv_sb = singles.tile([1, 2 * DIM], F16)
            nc.sync.dma_start(bkv_sb[:], bkv_d)

        # chunk-level feature-major activations; qnT/kvnT double-buffered so
        # next-chunk transposes run during this chunk's stage C
        chunk_sb = ctx.enter_context(tc.tile_pool(name="chunk", bufs=1))
        qkv_p = ctx.enter_context(tc.tile_pool(name="qkvT", bufs=2))
        # h-major sub-tile windows: window w (16 tokens), col = w*128 + h*16 + t
        QT = chunk_sb.tile([128, TC * HEADS], F16, tag="QT")
        KT = chunk_sb.tile([128, TC * HEADS], F16, tag="KT")
        VT = chunk_sb.tile([128, TC * HEADS], F16, tag="VT")

        raw_p = ctx.enter_context(tc.tile_pool(name="raw", bufs=16))
        st_p = ctx.enter_context(tc.tile_pool(name="stats", bufs=6))
        e_p = ctx.enter_context(tc.tile_pool(name="ebuf", bufs=3))
        em_p = ctx.enter_context(tc.tile_pool(name="embuf", bufs=3))
        p_p = ctx.enter_context(tc.tile_pool(name="pbuf", bufs=3))
        z_p = ctx.enter_context(tc.tile_pool(name="zbuf", bufs=6))
        l_p = ctx.enter_context(tc.tile_pool(name="lbuf", bufs=2))
        vb_p = ctx.enter_context(tc.tile_pool(name="vbuf", bufs=5))
        ctxT_p = ctx.enter_context(tc.tile_pool(name="ctxT", bufs=2))
        outsb_p = ctx.enter_context(tc.tile_pool(name="outsb", bufs=4))

        ps_tr = ctx.enter_context(tc.tile_pool(name="ps_tr", bufs=2, space="PSUM"))
        ps_mm = ctx.enter_context(tc.tile_pool(name="ps_mm", bufs=2, space="PSUM"))
        ps_s = ctx.enter_context(tc.tile_pool(name="ps_s", bufs=2, space="PSUM"))
        ps_o = ctx.enter_context(tc.tile_pool(name="ps_o", bufs=2, space="PSUM"))

        # per-chunk raw tiles, alive across two chunks (LN in chunk c-1's
        # emission, transposed at chunk c)
        raw_tiles = {}

        def emit_loads(c):
            if c >= NCH:
                return
            for it in range(TPC):
                tok0 = c * TC + it * TT
                for name, src in (("q", q_d), ("kv", kv_d)):
                    x = raw_p.tile([128, DIM], F16, tag="raw")
                    nc.sync.dma_start(x[:], src[tok0:tok0 + TT, :])
                    raw_tiles[(c, it, name)] = x

        def emit_ln(c, it, names=("q", "kv")):
            """LayerNorm tensors of tile (c, it) in place."""
            if c >= NCH:
                return
            for name in names:
                x = raw_tiles[(c, it, name)]
                stats = st_p.tile([128, 2, 6], F32, tag="bn")
                xg = x.rearrange("p (n f) -> p n f", n=2)
                for i in range(2):
                    nc.vector.bn_stats(out=stats[:, i, :], in_=xg[:, i, :])
                mv = st_p.tile([128, 2], F32, tag="mv")
                nc.vector.bn_aggr(out=mv[:], in_=stats[:])
                rstd = st_p.tile([128, 1], F32, tag="rstd")
                nc.scalar.activation(out=rstd[:], in_=mv[:, 1:2],
                                     func=mybir.ActivationFunctionType.Sqrt,
                                     bias=eps[:], scale=1.0)
                nc.vector.reciprocal(out=rstd[:], in_=rstd[:])
                nc.vector.tensor_scalar(out=x[:], in0=x[:],
                                        scalar1=mv[:, 0:1],
                                        scalar2=rstd[:],
                                        op0=mybir.AluOpType.subtract,
                                        op1=mybir.AluOpType.mult)

        chunk_qkv = {}

        def alloc_qkv(c):
            if c >= NCH:
                return
            qnT = qkv_p.tile([128, KT_F, TC], F16, tag="qnT")
            kvnT = qkv_p.tile([128, KT_F, TC], F16, tag="kvnT")
            chunk_qkv[c] = (qnT, kvnT)

        def emit_trans_tile(c, it, names=("q", "kv")):
            """PE transpose LN'd tile (c, it) to feature-major qnT/kvnT."""
            if c >= NCH:
                return
            qnT, kvnT = chunk_qkv[c]
            for name, dstT in (("q", qnT), ("kv", kvnT)):
                if name not in names:
                    continue
                x = raw_tiles.pop((c, it, name))
                tp = ps_tr.tile([128, KT_F, 128], F16, tag="tr")
                for f in range(KT_F):
                    nc.tensor.transpose(
                        tp[:, f, :], x[:, f * 128:(f + 1) * 128],
                        ident[:], )
                nc.vector.tensor_copy(
                    out=dstT[:, :, it * TT:(it + 1) * TT], in_=tp[:])

        def emit_projections(c):
            qnT, kvnT = chunk_qkv.pop(c)
            for m in range(HEADS):
                ps = ps_mm.tile([128, TC], F32, tag="mm")
                for k in range(KT_F):
                    nc.tensor.matmul(
                        ps[:], wq_sb[:, k, m * 128:(m + 1) * 128],
                        qnT[:, k, :], start=(k == 0),
                        stop=(k == KT_F - 1 and not with_bias_q))
                if with_bias_q:
                    nc.tensor.matmul(
                        ps[:], bq_sb[:, m * 128:(m + 1) * 128],
                        ones_row[:], start=False, stop=True)
                nc.scalar.copy(out=head_windows(QT, m, TC // TS), in_=ps[:])
            for m in range(2 * HEADS):
                ps = ps_mm.tile([128, TC], F32, tag="mm")
                for k in range(KT_F):
                    nc.tensor.matmul(
                        ps[:], wkv_sb[:, k, m * 128:(m + 1) * 128],
                        kvnT[:, k, :], start=(k == 0),
                        stop=(k == KT_F - 1 and not with_bias_kv))
                if with_bias_kv:
                    nc.tensor.matmul(
                        ps[:], bkv_sb[:, m * 128:(m + 1) * 128],
                        ones_row[:], start=False, stop=True)
                dst = KT if m < HEADS else VT
                nc.scalar.copy(out=head_windows(dst, m % HEADS, TC // TS),
                               in_=ps[:])

        def emit_attn_front(c, it):
            """Scores, V transposes, and the softmax chain up to P."""
            t0 = it * TT
            E = e_p.tile([128, SPT, 128], BF16, tag="e")
            EM = em_p.tile([128, SPT, 128], BF16, tag="em")
            P = p_p.tile([128, SPT, 128], F16, tag="p")
            z = z_p.tile([128, SPT], F32, tag="z")
            zr = z_p.tile([128, SPT], F32, tag="zr")
            Vb = vb_p.tile([128, SPT, 128], F16, tag="vb")
            st = {"E": E, "EM": EM, "P": P, "z": z, "zr": zr, "Vb": Vb,
                  "t0": t0}
            spss = []
            for b in range(2):
                sps = ps_s.tile([128, 4, 128], F32, tag="s")
                for s4 in range(4):
                    c0 = (t0 + (b * 4 + s4) * TS) * HEADS
                    nc.tensor.matmul(
                        sps[:, s4, :],
                        QT[:, c0:c0 + 128], KT[:, c0:c0 + 128],
                        start=True, stop=True, skip_group_check=True)
                spss.append(sps)
            vtp = ps_tr.tile([128, SPT, 128], F16, tag="tr")
            for s in range(SPT):
                c0 = (t0 + s * TS) * HEADS
                nc.tensor.transpose(
                    vtp[:, s, :], VT[:, c0:c0 + 128], ident[:])
            nc.scalar.copy(out=Vb[:], in_=vtp[:])
            for b in range(2):
                nc.scalar.activation(
                    out=E[:, b * 4:(b + 1) * 4, :], in_=spss[b],
                    func=mybir.ActivationFunctionType.Exp, scale=1.0)
                nc.vector.tensor_tensor(
                    out=EM[:, b * 4:(b + 1) * 4, :],
                    in0=E[:, b * 4:(b + 1) * 4, :], in1=mask[:],
                    op=mybir.AluOpType.mult)
                nc.vector.tensor_reduce(
                    out=z[:, b * 4:(b + 1) * 4],
                    in_=EM[:, b * 4:(b + 1) * 4, :],
                    op=mybir.AluOpType.add, axis=mybir.AxisListType.X)
                nc.vector.reciprocal(out=zr[:, b * 4:(b + 1) * 4],
                                     in_=z[:, b * 4:(b + 1) * 4])
                for s4 in range(4):
                    s = b * 4 + s4
                    if s4 % 2 == 0:
                        nc.vector.tensor_scalar(
                            out=P[:, s, :], in0=EM[:, s, :],
                            scalar1=zr[:, s:s + 1], scalar2=None,
                            op0=mybir.AluOpType.mult)
                    else:
                        nc.scalar.activation(
                            out=P[:, s, :], in_=EM[:, s, :],
                            func=mybir.ActivationFunctionType.Copy,
                            scale=zr[:, s:s + 1])
            return st

        def emit_attn_back(c, it, st):
            """P transposes, ctx matmuls, ctx copies; returns ctxT."""
            P, Vb = st["P"], st["Vb"]
            L = l_p.tile([128, SPT, 128], F16, tag="l")
            ctxT = ctxT_p.tile([128, HEADS, TT], F16, tag="ctxT")
            ptp = ps_tr.tile([128, SPT, 128], F16, tag="tr")
            for s in range(SPT):
                nc.tensor.transpose(ptp[:, s, :], P[:, s, :], ident[:])
            nc.vector.tensor_copy(out=L[:], in_=ptp[:])
            for b in range(2):
                cps = ps_s.tile([128, 4, 128], F32, tag="s")
                for s4 in range(4):
                    s = b * 4 + s4
                    nc.tensor.matmul(
                        cps[:, s4, :], Vb[:, s, :], L[:, s, :],
                        start=True, stop=True, skip_group_check=True)
                # batched reorder copy: src cols (h, s4, t) ->
                # ctxT[d][h][b*64 + s4*16 + t]
                src = bass.AP(tensor=cps.tensor, offset=cps.offset,
                              ap=[cps.ap[0], [TS, HEADS], [128, 4], [1, TS]])
                dst = bass.AP(tensor=ctxT.tensor,
                              offset=ctxT.offset + b * 64,
                              ap=[ctxT.ap[0], [TT, HEADS], [TS, 4], [1, TS]])
                if b == 0:
                    nc.scalar.copy(out=dst, in_=src)
                else:
                    nc.vector.tensor_copy(out=dst, in_=src)
            return ctxT

        def emit_oproj(c, it, ctxT):
            tok0 = c * TC + it * TT
            for oh in range(2):
                pso = ps_o.tile([128, 512], F32, tag="o")
                for h in range(HEADS):
                    nc.tensor.matmul(
                        pso[:], ctxT[:, h, :],
                        wo_sb[:, h, oh * 512:(oh + 1) * 512],
                        start=(h == 0), stop=(h == HEADS - 1))
                osb = outsb_p.tile([128, 512], F16, tag="osb")
                nc.scalar.copy(out=osb[:], in_=pso[:])
                nc.sync.dma_start(
                    out_d[tok0:tok0 + TT, oh * 512:(oh + 1) * 512], osb[:])

        # ---------------- main schedule ----------------
        # PE p-state warmup: a chain of identity transposes keeps the PE
        # continuously busy through the otherwise-idle DMA/LayerNorm
        # startup window so the first real matmuls start at full clock.
        for _ in range(24):
            wtp = ps_tr.tile([128, 128], F16, tag="tr")
            nc.tensor.transpose(wtp[:], ident[:], ident[:])
        # chunk 0 prologue: q tiles first so the Q-projection can start
        # while kv LayerNorms still run.
        emit_loads(0)
        emit_loads(1)
        emit_weight_loads()
        alloc_qkv(0)
        for it in range(TPC):
            emit_ln(0, it, names=("q",))
        for it in range(TPC):
            emit_trans_tile(0, it, names=("q",))
            emit_ln(0, it, names=("kv",))
        for it in range(TPC):
            emit_trans_tile(0, it, names=("kv",))

        for c in range(NCH):
            emit_loads(c + 2)
            alloc_qkv(c + 1)
            emit_projections(c)
            # next-chunk LayerNorms ride the stage-B window (DVE is idle
            # while PE runs projections), so stage-C DVE chains and the
            # interleaved next-chunk transposes never wait on them.
            for it in range(TPC):
                emit_ln(c + 1, it)
            # stage C, software-pipelined at tile depth 2: each tile's
            # P-dependent back half is emitted a full tile after its front,
            # so PE always has scores/V-transposes to run while the DVE
            # softmax chain completes. Next-chunk transposes are
            # interleaved as PE filler so the next chunk's projections
            # start immediately at the boundary.
            fronts = {}
            fronts[0] = emit_attn_front(c, 0)
            fronts[1] = emit_attn_front(c, 1)
            fronts[2] = emit_attn_front(c, 2)
            ctx0 = emit_attn_back(c, 0, fronts.pop(0))
            emit_trans_tile(c + 1, 0)
            fronts[3] = emit_attn_front(c, 3)
            ctx1 = emit_attn_back(c, 1, fronts.pop(1))
            emit_trans_tile(c + 1, 1)
            emit_oproj(c, 0, ctx0)
            ctx2 = emit_attn_back(c, 2, fronts.pop(2))
            emit_trans_tile(c + 1, 2)
            emit_oproj(c, 1, ctx1)
            ctx3 = emit_attn_back(c, 3, fronts.pop(3))
            emit_trans_tile(c + 1, 3)
            emit_oproj(c, 2, ctx2)
            emit_oproj(c, 3, ctx3)

    nc.finalize()
    return nc


def _host_mask():
    # h-major windows: row p = h*16+t, col q = g*16+t'; valid iff t == t'.
    # Tiled 4x horizontally for batched 4-subtile multiplies.
    m = np.zeros((TT, TT), np.float32)
    p = np.arange(TT)
    m[p[:, None] % TS == p[None, :] % TS] = 1.0
    return np.tile(m, (1, 4)).astype(ml_dtypes.bfloat16)


def kernel(q, kv, gamma_m, beta_m, gamma_l, beta_l, Wq, Wkv, Wo):
    q = np.asarray(q, np.float32)
    kv = np.asarray(kv, np.float32)
    bs, patch, _ = q.shape
    T_total = bs * patch
    T_core = T_total // NCORES

    scale = DHEAD ** (-0.5)
    # fold LN gamma into the projection weights, beta into bias vectors
    wq_eff = (np.asarray(Wq, np.float32) * np.asarray(gamma_m, np.float32)[None, :]) * scale
    bq = (np.asarray(Wq, np.float32) @ np.asarray(beta_m, np.float32)) * scale
    wkv_eff = np.asarray(Wkv, np.float32) * np.asarray(gamma_l, np.float32)[None, :]
    bkv = np.asarray(Wkv, np.float32) @ np.asarray(beta_l, np.float32)
    with_bias_q = bool(np.any(bq != 0.0))
    with_bias_kv = bool(np.any(bkv != 0.0))

    # kernel weight layout: [in, out], fp16
    wq_t = np.ascontiguousarray(wq_eff.T).astype(np.float16)
    wkv_t = np.ascontiguousarray(wkv_eff.T).astype(np.float16)
    wo_t = np.ascontiguousarray(np.asarray(Wo, np.float32).T).astype(np.float16)
    mask = _host_mask()

    nc = build_nc(T_core, with_bias_q, with_bias_kv)

    qf = q.reshape(T_total, DIM).astype(np.float16)
    kvf = kv.reshape(T_total, DIM).astype(np.float16)
    in_maps = []
    for i in range(NCORES):
        m = {
            "q": np.ascontiguousarray(qf[i * T_core:(i + 1) * T_core]),
            "kv": np.ascontiguousarray(kvf[i * T_core:(i + 1) * T_core]),
            "wq": wq_t, "wkv": wkv_t, "wo": wo_t, "mask": mask,
            "ident": np.eye(128, dtype=np.float16),
        }
        if with_bias_q:
            m["bq"] = bq.reshape(1, DIM).astype(np.float16)
        if with_bias_kv:
            m["bkv"] = bkv.reshape(1, 2 * DIM).astype(np.float16)
        in_maps.append(m)

    res = run_bass_kernel_spmd(nc, in_maps, list(range(NCORES)))
    global LAST_RESULTS
    LAST_RESULTS = res
    out = np.concatenate(
        [np.asarray(res.results[i]["out"], np.float32) for i in range(NCORES)],
        axis=0)
    return out.reshape(bs, patch, DIM)


LAST_RESULTS = None
